# revision 6
# baseline (speedup 1.0000x reference)
"""EMA (exponential moving average) kernel for Trainium2, 8 NeuronCores.

Problem: y[b,c,f,t] = w*x[b,c,f,t] + (1-w)*y[b,c,f,t-1], y[...,-1] = initial_state.
Shapes: mag_spec [8,2,257,6000] f32, initial_state [8,2,257,1] f32, weights [1] f32.

Sharding: data-parallel over batch. Core i gets b=i -> [514, 6000] rows,
each row an independent scan along time.

Per core: DMA-in x chunks -> ACT pre-scale (w*x) -> DVE tensor_tensor_scan
(state = (1-w)*state + w*x, the native first-order recurrence instruction)
-> DMA-out. Carry between time chunks is chained via initial=prev[:, -1:].
"""

import numpy as np

B, C, F, T = 8, 2, 257, 6000
R = C * F  # 514 rows per core
P = 128  # partitions
CH = 1500  # time-chunk width (6000 B per partition line per DMA)
N_CORES = 8

# knobs for test harness
TRACE = False
LAST_EXEC_NS = None
LAST_RESULTS = None
BUFS = 4

_cache = {}


def _build_bass(w: float, a: float):
    import concourse.bacc as bacc
    import concourse.mybir as mybir
    from concourse.tile import TileContext

    # Bacc (not Bass): its finalize() runs generate_event_semaphores, which
    # splits sync waits to satisfy the per-instruction wait-slot limits
    # (DMA and the scan format only have 1-2 slots).
    nc = bacc.Bacc(None)
    x_d = nc.dram_tensor("x", [R, T], mybir.dt.float32, kind="ExternalInput")
    init_d = nc.dram_tensor("init", [R, 1], mybir.dt.float32, kind="ExternalInput")
    y_d = nc.dram_tensor("y", [R, T], mybir.dt.float32, kind="ExternalOutput")

    n_chunks = T // CH
    assert T % CH == 0

    # The scan ISA format (S2S2D2_STT) has very few semaphore-wait slots, so
    # the scan instruction must carry at most ONE cross-engine wait (the ACT
    # prescale). Therefore:
    #  - the scan output aliases the x tile: its WAR on ACT's read of x is
    #    covered by the same ACT-sem wait as "xw ready" (same ACT op)
    #  - the chunk-0 initial goes through a DVE tensor_copy so the scan's
    #    dep on it is same-engine program order
    #  - the chunk>0 initial is the previous scan's output column (same engine)
    with TileContext(nc) as tc:
        with (
            tc.tile_pool(name="const", bufs=1) as cpool,
            tc.tile_pool(name="work", bufs=BUFS) as pool,
        ):
            a_tile = cpool.tile([P, CH], mybir.dt.float32)
            nc.vector.memset(a_tile[:], a)
            for blk in range(0, R, P):
                rows = min(P, R - blk)
                init_t = pool.tile([P, 1], mybir.dt.float32, tag="init")
                nc.sync.dma_start(
                    out=init_t[:rows], in_=init_d[blk : blk + rows, :]
                )
                init_v = pool.tile([P, 1], mybir.dt.float32, tag="initv")
                nc.vector.tensor_copy(out=init_v[:rows], in_=init_t[:rows])
                prev = init_v[:rows, 0:1]
                for c in range(n_chunks):
                    lo = c * CH
                    x_t = pool.tile([P, CH], mybir.dt.float32, tag="x")
                    nc.sync.dma_start(
                        out=x_t[:rows], in_=x_d[blk : blk + rows, lo : lo + CH]
                    )
                    # Prescale on the DVE itself, then scan in place: the
                    # scan's deps are then DVE-self (prescale + carry) plus at
                    # most one DMA WAR — the scan ISA format only has 2
                    # sync-wait slots.
                    xw_t = pool.tile([P, CH], mybir.dt.float32, tag="xw")
                    nc.vector.tensor_scalar_mul(xw_t[:rows], x_t[:rows], w)
                    nc.vector.tensor_tensor_scan(
                        out=xw_t[:rows],
                        data0=a_tile[:rows],
                        data1=xw_t[:rows],
                        initial=prev,
                        op0=mybir.AluOpType.mult,
                        op1=mybir.AluOpType.add,
                    )
                    nc.sync.dma_start(
                        out=y_d[blk : blk + rows, lo : lo + CH], in_=xw_t[:rows]
                    )
                    prev = xw_t[:rows, CH - 1 : CH]
    nc.finalize()
    return nc


def kernel(mag_spec, initial_state, weights):
    global LAST_EXEC_NS, LAST_RESULTS
    from concourse.bass_utils import run_bass_kernel_spmd

    mag_spec = np.asarray(mag_spec, dtype=np.float32)
    initial_state = np.asarray(initial_state, dtype=np.float32)
    w = float(np.clip(np.asarray(weights, dtype=np.float32), 0.0, 1.0).reshape(-1)[0])
    a = float(np.float32(1.0) - np.float32(w))

    key = (w, a, CH, BUFS)
    if key not in _cache:
        _cache[key] = _build_bass(w, a)
    nc = _cache[key]

    in_maps = []
    for i in range(N_CORES):
        in_maps.append(
            {
                "x": np.ascontiguousarray(mag_spec[i].reshape(R, T)),
                "init": np.ascontiguousarray(initial_state[i].reshape(R, 1)),
            }
        )

    res = run_bass_kernel_spmd(nc, in_maps, list(range(N_CORES)), trace=TRACE)
    LAST_EXEC_NS = res.exec_time_ns
    LAST_RESULTS = res
    out = np.stack(
        [res.results[i]["y"].reshape(C, F, T) for i in range(N_CORES)], axis=0
    )
    return out


# revision 7
# speedup vs baseline: 1.0757x; 1.0757x over previous
"""EMA (exponential moving average) kernel for Trainium2, 8 NeuronCores.

Problem: y[b,c,f,t] = w*x[b,c,f,t] + (1-w)*y[b,c,f,t-1], y[...,-1] = initial_state.
Shapes: mag_spec [8,2,257,6000] f32, initial_state [8,2,257,1] f32, weights [1] f32.

Sharding: data-parallel over batch. Core i gets b=i -> [514, 6000] rows,
each row an independent scan along time.

Per core: DMA-in x chunks -> ACT pre-scale (w*x) -> DVE tensor_tensor_scan
(state = (1-w)*state + w*x, the native first-order recurrence instruction)
-> DMA-out. Carry between time chunks is chained via initial=prev[:, -1:].
"""

import numpy as np

B, C, F, T = 8, 2, 257, 6000
R = C * F  # 514 rows per core
P = 128  # partitions
CH = 1500  # time-chunk width (6000 B per partition line per DMA)
N_CORES = 8

# knobs for test harness
TRACE = False
LAST_EXEC_NS = None
LAST_RESULTS = None
BUFS = 4

_cache = {}


def _build_bass(w: float, a: float):
    import concourse.bacc as bacc
    import concourse.mybir as mybir
    from concourse.tile import TileContext

    # Bacc (not Bass): its finalize() runs generate_event_semaphores, which
    # splits sync waits to satisfy the per-instruction wait-slot limits
    # (DMA and the scan format only have 1-2 slots).
    nc = bacc.Bacc(None)
    x_d = nc.dram_tensor("x", [R, T], mybir.dt.float32, kind="ExternalInput")
    init_d = nc.dram_tensor("init", [R, 1], mybir.dt.float32, kind="ExternalInput")
    y_d = nc.dram_tensor("y", [R, T], mybir.dt.float32, kind="ExternalOutput")

    n_chunks = T // CH
    assert T % CH == 0

    # The scan ISA format (S2S2D2_STT) has very few semaphore-wait slots, so
    # the scan instruction must carry at most ONE cross-engine wait (the ACT
    # prescale). Therefore:
    #  - the scan output aliases the x tile: its WAR on ACT's read of x is
    #    covered by the same ACT-sem wait as "xw ready" (same ACT op)
    #  - the chunk-0 initial goes through a DVE tensor_copy so the scan's
    #    dep on it is same-engine program order
    #  - the chunk>0 initial is the previous scan's output column (same engine)
    with TileContext(nc) as tc:
        with (
            tc.tile_pool(name="const", bufs=1) as cpool,
            tc.tile_pool(name="work", bufs=BUFS) as pool,
        ):
            a_tile = cpool.tile([P, CH], mybir.dt.float32)
            nc.vector.memset(a_tile[:], a)
            for blk in range(0, R, P):
                rows = min(P, R - blk)
                init_t = pool.tile([P, 1], mybir.dt.float32, tag="init")
                nc.sync.dma_start(
                    out=init_t[:rows], in_=init_d[blk : blk + rows, :]
                )
                init_v = pool.tile([P, 1], mybir.dt.float32, tag="initv")
                nc.vector.tensor_copy(out=init_v[:rows], in_=init_t[:rows])
                prev = init_v[:rows, 0:1]
                for c in range(n_chunks):
                    lo = c * CH
                    x_t = pool.tile([P, CH], mybir.dt.float32, tag="x")
                    nc.sync.dma_start(
                        out=x_t[:rows], in_=x_d[blk : blk + rows, lo : lo + CH]
                    )
                    # Prescale on ACT so the DVE only runs the scans (the
                    # scan is the serial bottleneck at ~3 cycles/column).
                    # Bacc's generate_event_semaphores legalizes the scan's
                    # multi-wait into event-semaphore instructions.
                    xw_t = pool.tile([P, CH], mybir.dt.float32, tag="xw")
                    nc.scalar.mul(xw_t[:rows], x_t[:rows], w)
                    nc.vector.tensor_tensor_scan(
                        out=xw_t[:rows],
                        data0=a_tile[:rows],
                        data1=xw_t[:rows],
                        initial=prev,
                        op0=mybir.AluOpType.mult,
                        op1=mybir.AluOpType.add,
                    )
                    nc.sync.dma_start(
                        out=y_d[blk : blk + rows, lo : lo + CH], in_=xw_t[:rows]
                    )
                    prev = xw_t[:rows, CH - 1 : CH]
    nc.finalize()
    return nc


def kernel(mag_spec, initial_state, weights):
    global LAST_EXEC_NS, LAST_RESULTS
    from concourse.bass_utils import run_bass_kernel_spmd

    mag_spec = np.asarray(mag_spec, dtype=np.float32)
    initial_state = np.asarray(initial_state, dtype=np.float32)
    w = float(np.clip(np.asarray(weights, dtype=np.float32), 0.0, 1.0).reshape(-1)[0])
    a = float(np.float32(1.0) - np.float32(w))

    key = (w, a, CH, BUFS)
    if key not in _cache:
        _cache[key] = _build_bass(w, a)
    nc = _cache[key]

    in_maps = []
    for i in range(N_CORES):
        in_maps.append(
            {
                "x": np.ascontiguousarray(mag_spec[i].reshape(R, T)),
                "init": np.ascontiguousarray(initial_state[i].reshape(R, 1)),
            }
        )

    res = run_bass_kernel_spmd(nc, in_maps, list(range(N_CORES)), trace=TRACE)
    LAST_EXEC_NS = res.exec_time_ns
    LAST_RESULTS = res
    out = np.stack(
        [res.results[i]["y"].reshape(C, F, T) for i in range(N_CORES)], axis=0
    )
    return out
